# revision 3
# baseline (speedup 1.0000x reference)
"""DDI regularizer loss kernel for 8 Trainium2 NeuronCores.

reference semantics:
    b = (ddi > 0); S = max(b, b.T) with zero diagonal; U = triu(S, k=1)
    normalizer = max(U.sum(), 1.0)
    xu = drug_probs @ U; penalties = sum(xu * drug_probs, axis=1) / normalizer
    return penalties.mean()

Identity used here:
    mean_i(x_i^T U x_i) = <U, X^T X> / B
so the kernel computes G = X^T X only on upper-triangular 128x512 tiles
(contraction over the batch is the natural PE layout - no transposes of X),
masks each G tile with U's tile (built on device from fp8 ddi slices) and
reduces.  40 real tiles + 8 dummy slots are distributed 6-per-core across the
8 cores; each core returns per-partition partial sums of (U*G) and of U, and
the host combines 8 tiny vectors into the final scalar.

The matmuls run in fp8 e5m2 with DoubleRow packing (two 128-row batch chunks
per matmul, fp32 PSUM accumulation).  Pipeline layout (v2):
  - X stream rides the sync HWDGE queue alone, host-packed so every chunk is
    one 2560B descriptor per partition; ddi/thr ride the gpsimd SWDGE queue
    and are issued first so they never delay the X stream.
  - ddi slices are fp8 (sign comparison only needs the sign bit).
  - 4 warmup matmuls on a memset tile raise the PE HAM clock to 8/8 before
    real data lands.
  - The binarized-mirror transposes run mid-stream (after chunk 5), their
    PSUM->SBUF copies on the otherwise idle scalar engine; sel/binA compares
    run on gpsimd; DVE only does binB binarize + max/mult + final reduces.
  - The normalizer sum(U) is accumulated on the PE via a ones-vector matmul
    instead of 6 DVE reduces.
"""

import sys

for _p in ("/opt/trn_rl_repo", "/root/.axon_site/_ro/trn_rl_repo"):
    if _p not in sys.path:
        sys.path.insert(0, _p)

import numpy as np
import ml_dtypes

B, D = 4096, 2048
NBLK = 128  # lhs row-block width
NCOL = 512  # rhs col-block width
NSLOT = 6  # tile slots per core
NWARM = 4  # PE clock warmup matmuls
TRANSPOSE_AT = 5  # emit mirror transposes after this many X chunks

# (J, [row-block indices; -1 = dummy slot]) per core.  Tile (i, J) covers
# G[128i:128i+128, 512J:512J+512]; it exists iff i <= 4J+3 (touches the
# strict upper triangle).
CORE_ASSIGN = [
    (3, [0, 1, 2, 3, 4, 5]),
    (3, [6, 7, 8, 9, 10, 11]),
    (3, [12, 13, 14, 15, -1, -1]),
    (2, [0, 1, 2, 3, 4, 5]),
    (2, [6, 7, 8, 9, 10, 11]),
    (1, [0, 1, 2, 3, 4, 5]),
    (1, [6, 7, -1, -1, -1, -1]),
    (0, [0, 1, 2, 3, -1, -1]),
]

NIN = NCOL + NBLK * NSLOT  # 1280 columns in the merged X input
NK = B // 256  # two 128-row chunks per DoubleRow matmul

_CACHE = {}


def _build():
    import concourse.bass as bass
    import concourse.mybir as mybir
    from concourse import bacc
    from concourse.tile import TileContext

    f32 = mybir.dt.float32
    bf16 = mybir.dt.bfloat16
    fp8 = mybir.dt.float8e5
    op = mybir.AluOpType

    nc = bacc.Bacc("TRN2", target_bir_lowering=False, debug=False, num_devices=8)

    # xin: chunk-major DoubleRow layout - row 128k+p holds the two batch rows
    # 256k+p and 256k+128+p back to back (2560 contiguous bytes / partition).
    xin_d = nc.dram_tensor("xin", [NK * 128, 2 * NIN], fp8, kind="ExternalInput")
    ddiA_d = nc.dram_tensor("ddiA", [NBLK, NCOL * NSLOT], fp8, kind="ExternalInput")
    ddiB_d = nc.dram_tensor("ddiB", [128, 4 * NBLK * NSLOT], fp8, kind="ExternalInput")
    thr_d = nc.dram_tensor("thr", [128, NSLOT], f32, kind="ExternalInput")
    out_d = nc.dram_tensor("out", [128, NSLOT + 1], f32, kind="ExternalOutput")

    with TileContext(nc) as tc:
        with (
            tc.tile_pool(name="const", bufs=1) as cpool,
            tc.tile_pool(name="masks", bufs=NSLOT) as mpool,
            tc.tile_pool(name="io", bufs=10) as iopool,
            tc.tile_pool(name="psum", bufs=NSLOT, space="PSUM") as ppool,
            tc.tile_pool(name="tpp", bufs=2, space="PSUM") as tppool,
            tc.tile_pool(name="scr", bufs=3) as spool,
        ):
            # --- ddi/thr loads first on the SWDGE queue (gpsimd) ---
            ddiA_sb = cpool.tile([NBLK, NCOL * NSLOT], fp8, tag="ddiA")
            nc.gpsimd.dma_start(out=ddiA_sb, in_=ddiA_d.ap())
            ddiB_sb = cpool.tile([128, 4, NBLK * NSLOT], fp8, tag="ddiB")
            nc.gpsimd.dma_start(out=ddiB_sb, in_=ddiB_d.ap())
            thr_sb = cpool.tile([128, NSLOT], f32, tag="thr")
            nc.gpsimd.dma_start(out=thr_sb, in_=thr_d.ap())

            # --- vector: warmup source + tiny consts (all pure writes) ---
            wsrc = cpool.tile([128, 2, NCOL + NBLK], fp8, tag="wsrc")
            nc.vector.memset(wsrc, 0.0)
            ones = cpool.tile([128, 1], bf16, tag="ones")
            nc.vector.memset(ones, 1.0)
            out_sb = cpool.tile([128, NSLOT + 1], f32, tag="out")
            nc.vector.memset(out_sb[:, NSLOT : NSLOT + 1], 0.0)

            # --- PE clock warmup: a few matmuls on the memset tile ---
            wps = tppool.tile([128, NCOL], f32, tag="tp", name="warm")
            for w in range(NWARM):
                nc.tensor.matmul(
                    out=wps,
                    lhsT=wsrc[:, :, NCOL : NCOL + NBLK],
                    rhs=wsrc[:, :, 0:NCOL],
                    start=True,
                    stop=True,
                    perf_mode=mybir.MatmulPerfMode.DoubleRow,
                )

            # --- constants from native iota (no DMA) ---
            iota = cpool.tile([128, NCOL], f32, tag="iota")
            nc.gpsimd.iota(
                iota,
                pattern=[[1, NCOL]],
                base=0,
                channel_multiplier=0,
                allow_small_or_imprecise_dtypes=True,
            )
            iotap = cpool.tile([128, 1], f32, tag="iotap")
            nc.gpsimd.iota(
                iotap,
                pattern=[[1, 1]],
                base=0,
                channel_multiplier=1,
                allow_small_or_imprecise_dtypes=True,
            )
            idn = cpool.tile([128, NBLK], bf16, tag="idn")
            nc.vector.tensor_scalar(
                out=idn, in0=iota[:, :NBLK], scalar1=iotap, scalar2=None,
                op0=op.is_equal,
            )

            # binarize the mirror blocks on DVE (fp8 in, bf16 out)
            binB = cpool.tile([128, 4, NBLK * NSLOT], bf16, tag="binB")
            for s in range(4):
                nc.vector.tensor_scalar(
                    out=binB[:, s], in0=ddiB_sb[:, s], scalar1=0.0, scalar2=None,
                    op0=op.is_gt,
                )

            # sel/binA compares on gpsimd (DVE stays free for binB + masks)
            sels = []
            binAs = []
            for t in range(NSLOT):
                sel = spool.tile([128, NCOL], bf16, tag="sel", name=f"sel{t}")
                nc.gpsimd.tensor_scalar(
                    out=sel, in0=iota, scalar1=thr_sb[:, t : t + 1],
                    scalar2=None, op0=op.is_gt,
                )
                sels.append(sel)
                binA = spool.tile([128, NCOL], bf16, tag="binA", name=f"binA{t}")
                nc.gpsimd.tensor_scalar(
                    out=binA, in0=ddiA_sb[:, t * NCOL : (t + 1) * NCOL],
                    scalar1=0.0, scalar2=None, op0=op.is_gt,
                )
                binAs.append(binA)

            # --- G tiles: accumulating matmuls, k-outer so the X stream is
            # consumed strictly in order; mirror transposes dropped in
            # mid-stream so they never gate the head or tail ---
            psums = [
                ppool.tile([128, NCOL], f32, tag="gps", name=f"gps{t}")
                for t in range(NSLOT)
            ]
            ddiBT = [
                cpool.tile([NBLK, NCOL], bf16, tag=f"ddiBT{t}", name=f"ddiBT{t}")
                for t in range(NSLOT)
            ]
            xin_ap = xin_d.ap().rearrange("(k p) (i c) -> k p i c", p=128, i=2)
            for k in range(NK):
                xt = iopool.tile([128, 2, NIN], fp8, tag="xt")
                nc.sync.dma_start(out=xt, in_=xin_ap[k])
                for t in range(NSLOT):
                    c0 = NCOL + t * NBLK
                    nc.tensor.matmul(
                        out=psums[t],
                        lhsT=xt[:, :, c0 : c0 + NBLK],
                        rhs=xt[:, :, 0:NCOL],
                        start=(k == 0),
                        stop=(k == NK - 1),
                        perf_mode=mybir.MatmulPerfMode.DoubleRow,
                    )
                if k == TRANSPOSE_AT:
                    # transpose the binarized mirror tiles on the PE; copies
                    # to SBUF ride the otherwise idle scalar engine
                    for t in range(NSLOT):
                        for s in range(4):
                            pst = tppool.tile(
                                [128, NBLK], bf16, tag="tp", name=f"tp{t}_{s}"
                            )
                            nc.tensor.transpose(
                                out=pst,
                                in_=binB[:, s, t * NBLK : (t + 1) * NBLK],
                                identity=idn,
                            )
                            nc.scalar.copy(
                                out=ddiBT[t][:, s * NBLK : (s + 1) * NBLK], in_=pst
                            )

            # masks on DVE, overlapped with the matmul phase:
            # U_tile = max(A>0, B.T>0) * (col > row)
            masks = []
            for t in range(NSLOT):
                mraw = spool.tile([128, NCOL], bf16, tag="mraw")
                nc.vector.tensor_tensor(out=mraw, in0=binAs[t], in1=ddiBT[t], op=op.max)
                mask = mpool.tile([128, NCOL], bf16, tag="mask")
                nc.vector.tensor_tensor(out=mask, in0=mraw, in1=sels[t], op=op.mult)
                masks.append(mask)

            # normalizer partials sum(U) accumulate on the PE: ones^T @ mask
            nps = tppool.tile([1, NCOL], f32, tag="tp", name="normps")
            for t in range(NSLOT):
                nc.tensor.matmul(
                    out=nps,
                    lhsT=ones,
                    rhs=masks[t],
                    start=(t == 0),
                    stop=(t == NSLOT - 1),
                )
            njunk = spool.tile([1, NCOL], f32, tag="njunk")
            nc.vector.tensor_scalar(
                out=njunk, in0=nps, scalar1=1.0, scalar2=None, op0=op.mult,
                op1=op.add,
                accum_out=out_sb[0:1, NSLOT : NSLOT + 1],
            )

            # --- masked reduction: sum(G * mask), one fused op per slot ---
            for t in range(NSLOT):
                gjunk = spool.tile([128, NCOL], f32, tag="gjunk")
                nc.vector.scalar_tensor_tensor(
                    out=gjunk, in0=psums[t], scalar=1.0, in1=masks[t],
                    op0=op.mult, op1=op.mult,
                    accum_out=out_sb[:, t : t + 1],
                )

            nc.gpsimd.dma_start(out=out_d.ap(), in_=out_sb)

    nc.compile()
    return nc


def _in_maps(drug_probs, ddi_matrix):
    fp8 = ml_dtypes.float8_e5m2
    xq = drug_probs.astype(fp8)
    db = ddi_matrix.astype(fp8)
    zero_x = np.zeros((B, NBLK), dtype=fp8)
    zero_a = np.zeros((NBLK, NCOL), dtype=fp8)
    zero_b = np.zeros((NCOL, NBLK), dtype=fp8)
    maps = []
    for J, slots in CORE_ASSIGN:
        xin = np.concatenate(
            [xq[:, J * NCOL : (J + 1) * NCOL]]
            + [xq[:, i * NBLK : (i + 1) * NBLK] if i >= 0 else zero_x for i in slots],
            axis=1,
        )
        # chunk-major DoubleRow packing: [4096, NIN] -> [16*128, 2*NIN]
        xin = (
            xin.reshape(NK, 2, 128, NIN)
            .transpose(0, 2, 1, 3)
            .reshape(NK * 128, 2 * NIN)
        )
        ddiA = np.concatenate(
            [
                db[i * NBLK : (i + 1) * NBLK, J * NCOL : (J + 1) * NCOL]
                if i >= 0
                else zero_a
                for i in slots
            ],
            axis=1,
        )
        # mirror blocks: [512, 128] per slot -> [128(p), 4(s), 128(c)], packed
        # per-partition contiguous: ddiB[p, s*768 + t*128 + c] = B_t[s*128+p, c]
        ddiB = np.stack(
            [
                (
                    db[J * NCOL : (J + 1) * NCOL, i * NBLK : (i + 1) * NBLK]
                    if i >= 0
                    else zero_b
                ).reshape(4, 128, NBLK)
                for i in slots
            ],
            axis=0,
        )  # [t, s, p, c]
        ddiB = ddiB.transpose(2, 1, 0, 3).reshape(128, 4 * NSLOT * NBLK)
        p = np.arange(128, dtype=np.float32)[:, None]
        thr = np.concatenate(
            [
                p + np.float32(i * NBLK - J * NCOL)
                if i >= 0
                else np.full((128, 1), 1e9, np.float32)
                for i in slots
            ],
            axis=1,
        )
        maps.append(
            {
                "xin": np.ascontiguousarray(xin),
                "ddiA": np.ascontiguousarray(ddiA),
                "ddiB": np.ascontiguousarray(ddiB),
                "thr": np.ascontiguousarray(thr),
            }
        )
    return maps


def kernel(drug_probs, ddi_matrix, **_run_kwargs):
    from concourse.bass_utils import run_bass_kernel_spmd

    if "nc" not in _CACHE:
        _CACHE["nc"] = _build()
    nc = _CACHE["nc"]

    maps = _in_maps(np.asarray(drug_probs), np.asarray(ddi_matrix))
    res = run_bass_kernel_spmd(nc, maps, list(range(8)), **_run_kwargs)
    _CACHE["last_result"] = res

    gsum = 0.0
    msum = 0.0
    for core_out in res.results:
        o = core_out["out"].astype(np.float64)
        gsum += o[:, :NSLOT].sum()
        msum += o[:, NSLOT].sum()
    normalizer = max(msum, 1.0)
    return np.asarray(gsum / (B * normalizer), dtype=np.float32)


# revision 4
# speedup vs baseline: 1.8991x; 1.8991x over previous
"""DDI regularizer loss kernel for 8 Trainium2 NeuronCores.

reference semantics:
    b = (ddi > 0); S = max(b, b.T) with zero diagonal; U = triu(S, k=1)
    normalizer = max(U.sum(), 1.0)
    xu = drug_probs @ U; penalties = sum(xu * drug_probs, axis=1) / normalizer
    return penalties.mean()

Identity used here:
    mean_i(x_i^T U x_i) = <U, X^T X> / B
so the kernel computes G = X^T X only on upper-triangular 128x512 tiles
(contraction over the batch is the natural PE layout), masks each G tile with
U's tile (built on device from ddi slices) and reduces.  40 real tiles + 8
dummy slots are distributed 6-per-core across the 8 cores; each core returns
per-partition partial sums of (U*G) and of U, and the host combines 8 tiny
vectors into the final scalar.

The matmuls run in fp8 e5m2 with DoubleRow packing (two 128-row batch chunks
per matmul, fp32 PSUM accumulation).  Pipeline (v2):
  - X stream on the sync HWDGE queue, host-packed so every chunk is one
    2560B descriptor per partition; a few warmup matmuls on a memset tile
    raise the PE HAM clock before real data lands.
  - mirror ddi blocks arrive TRANSPOSED via the xbar DMA-transpose path on
    the scalar HWDGE queue (no PE transposes, no PSUM staging); the A-side
    ddi rides a casting SWDGE (gpsimd) DMA as fp8-over-the-wire -> bf16.
  - binarize is scalar-engine Sign (-1/0/1); the DVE combine
    mask = relu(max(signA, signB^T)) * sel fixes the -1 case in one fused
    scalar_tensor_tensor op.  The normalizer partial sum(U) is a scalar
    engine Copy-with-accum.  DVE never touches fp8 and gpsimd never runs
    elementwise ops (both are ~16x slow paths).
"""

import sys

for _p in ("/opt/trn_rl_repo", "/root/.axon_site/_ro/trn_rl_repo"):
    if _p not in sys.path:
        sys.path.insert(0, _p)

import numpy as np
import ml_dtypes

B, D = 4096, 2048
NBLK = 128  # lhs row-block width
NCOL = 512  # rhs col-block width
NSLOT = 6  # tile slots per core
NWARM = 4  # PE clock warmup matmuls
NK = B // 256  # two 128-row chunks per DoubleRow matmul

# (J, [row-block indices; -1 = dummy slot]) per core.  Tile (i, J) covers
# G[128i:128i+128, 512J:512J+512]; it exists iff i <= 4J+3 (touches the
# strict upper triangle).
CORE_ASSIGN = [
    (3, [0, 1, 2, 3, 4, 5]),
    (3, [6, 7, 8, 9, 10, 11]),
    (3, [12, 13, 14, 15, -1, -1]),
    (2, [0, 1, 2, 3, 4, 5]),
    (2, [6, 7, 8, 9, 10, 11]),
    (1, [0, 1, 2, 3, 4, 5]),
    (1, [6, 7, -1, -1, -1, -1]),
    (0, [0, 1, 2, 3, -1, -1]),
]

NIN = NCOL + NBLK * NSLOT  # 1280 columns in the merged X input

_CACHE = {}


def _build():
    import concourse.bass as bass
    import concourse.mybir as mybir
    from concourse import bacc
    from concourse.tile import TileContext

    f32 = mybir.dt.float32
    bf16 = mybir.dt.bfloat16
    fp8 = mybir.dt.float8e5
    op = mybir.AluOpType
    act = mybir.ActivationFunctionType

    nc = bacc.Bacc("TRN2", target_bir_lowering=False, debug=False, num_devices=8)

    # xin: chunk-major DoubleRow layout - row 128k+p holds the two batch rows
    # 256k+p and 256k+128+p back to back (2560 contiguous bytes / partition).
    xin_d = nc.dram_tensor("xin", [NK * 128, 2 * NIN], fp8, kind="ExternalInput")
    ddiA_d = nc.dram_tensor("ddiA", [NBLK, NCOL * NSLOT], fp8, kind="ExternalInput")
    # mirror blocks, natural [512, 128] orientation per slot (for xbar transpose)
    ddiB_d = nc.dram_tensor("ddiB", [NCOL, NBLK * NSLOT], bf16, kind="ExternalInput")
    thr_d = nc.dram_tensor("thr", [128, NSLOT], f32, kind="ExternalInput")
    out_d = nc.dram_tensor("out", [128, 2 * NSLOT], f32, kind="ExternalOutput")

    with TileContext(nc) as tc:
        with (
            tc.tile_pool(name="const", bufs=1) as cpool,
            tc.tile_pool(name="masks", bufs=NSLOT) as mpool,
            tc.tile_pool(name="io", bufs=10) as iopool,
            tc.tile_pool(name="psum", bufs=NSLOT, space="PSUM") as ppool,
            tc.tile_pool(name="tpp", bufs=1, space="PSUM") as tppool,
            tc.tile_pool(name="scr", bufs=3) as spool,
        ):
            # --- mirror blocks arrive pre-transposed via xbar DMA on the
            # scalar HWDGE queue (issued first; xin rides sync) ---
            ddiBT_raw = cpool.tile([128, NSLOT, NCOL], bf16, tag="ddiBTr")
            for t in range(NSLOT):
                nc.scalar.dma_start_transpose(
                    out=ddiBT_raw[:, t],
                    in_=ddiB_d.ap()[:, t * NBLK : (t + 1) * NBLK],
                )

            # --- A-side ddi as casting SWDGE DMA (fp8 on the wire) + thr ---
            ddiA_sb = cpool.tile([NBLK, NCOL * NSLOT], bf16, tag="ddiA")
            nc.gpsimd.dma_start(out=ddiA_sb, in_=ddiA_d.ap())
            thr_sb = cpool.tile([128, NSLOT], f32, tag="thr")
            nc.gpsimd.dma_start(out=thr_sb, in_=thr_d.ap())

            # --- vector: warmup source (pure write) ---
            wsrc = cpool.tile([128, 2, NCOL + NBLK], fp8, tag="wsrc")
            nc.vector.memset(wsrc, 0.0)

            # --- PE clock warmup on the memset tile ---
            wps = tppool.tile([128, NCOL], f32, tag="tp", name="warm")
            for w in range(NWARM):
                nc.tensor.matmul(
                    out=wps,
                    lhsT=wsrc[:, :, NCOL : NCOL + NBLK],
                    rhs=wsrc[:, :, 0:NCOL],
                    start=True,
                    stop=True,
                    perf_mode=mybir.MatmulPerfMode.DoubleRow,
                )

            # column-index iota for the triangular sel mask
            iota = cpool.tile([128, NCOL], f32, tag="iota")
            nc.gpsimd.iota(
                iota,
                pattern=[[1, NCOL]],
                base=0,
                channel_multiplier=0,
                allow_small_or_imprecise_dtypes=True,
            )

            # binarize to sign (-1/0/1) on the scalar engine
            binA = cpool.tile([NBLK, NCOL * NSLOT], bf16, tag="binA")
            nc.scalar.activation(out=binA, in_=ddiA_sb, func=act.Sign)
            binBT = cpool.tile([128, NSLOT, NCOL], bf16, tag="binBT")
            for t in range(NSLOT):
                nc.scalar.activation(
                    out=binBT[:, t], in_=ddiBT_raw[:, t], func=act.Sign
                )

            # --- G tiles: accumulating matmuls, k-outer so the X stream is
            # consumed strictly in order ---
            psums = [
                ppool.tile([128, NCOL], f32, tag="gps", name=f"gps{t}")
                for t in range(NSLOT)
            ]
            xin_ap = xin_d.ap().rearrange("(k p) (i c) -> k p i c", p=128, i=2)
            for k in range(NK):
                xt = iopool.tile([128, 2, NIN], fp8, tag="xt")
                nc.sync.dma_start(out=xt, in_=xin_ap[k])
                for t in range(NSLOT):
                    c0 = NCOL + t * NBLK
                    nc.tensor.matmul(
                        out=psums[t],
                        lhsT=xt[:, :, c0 : c0 + NBLK],
                        rhs=xt[:, :, 0:NCOL],
                        start=(k == 0),
                        stop=(k == NK - 1),
                        perf_mode=mybir.MatmulPerfMode.DoubleRow,
                    )

            # masks on DVE, overlapped with the matmul phase:
            # U_tile = relu(max(signA, signB^T)) * (col > row)
            out_sb = cpool.tile([128, 2 * NSLOT], f32, tag="out")
            masks = []
            for t in range(NSLOT):
                sel = spool.tile([128, NCOL], bf16, tag="sel")
                nc.vector.tensor_scalar(
                    out=sel, in0=iota, scalar1=thr_sb[:, t : t + 1],
                    scalar2=None, op0=op.is_gt,
                )
                mraw = spool.tile([128, NCOL], bf16, tag="mraw")
                nc.vector.tensor_tensor(
                    out=mraw, in0=binA[:, t * NCOL : (t + 1) * NCOL],
                    in1=binBT[:, t], op=op.max,
                )
                mask = mpool.tile([128, NCOL], bf16, tag="mask")
                nc.vector.scalar_tensor_tensor(
                    out=mask, in0=mraw, scalar=0.0, in1=sel,
                    op0=op.max, op1=op.mult,
                )
                masks.append(mask)
                # normalizer partial sum(U) on the scalar engine
                mjunk = spool.tile([128, NCOL], bf16, tag="mjunk")
                nc.scalar.activation(
                    out=mjunk, in_=mask, func=act.Copy,
                    accum_out=out_sb[:, NSLOT + t : NSLOT + t + 1],
                )

            # --- masked reduction: sum(G * mask), one fused op per slot ---
            for t in range(NSLOT):
                gjunk = spool.tile([128, NCOL], f32, tag="gjunk")
                nc.vector.scalar_tensor_tensor(
                    out=gjunk, in0=psums[t], scalar=1.0, in1=masks[t],
                    op0=op.mult, op1=op.mult,
                    accum_out=out_sb[:, t : t + 1],
                )

            nc.gpsimd.dma_start(out=out_d.ap(), in_=out_sb)

    nc.compile()
    return nc


def _in_maps(drug_probs, ddi_matrix):
    fp8 = ml_dtypes.float8_e5m2
    bf16 = ml_dtypes.bfloat16
    xq = drug_probs.astype(fp8)
    db8 = ddi_matrix.astype(fp8)
    db16 = ddi_matrix.astype(bf16)
    zero_x = np.zeros((B, NBLK), dtype=fp8)
    zero_a = np.zeros((NBLK, NCOL), dtype=fp8)
    zero_b = np.zeros((NCOL, NBLK), dtype=bf16)
    maps = []
    for J, slots in CORE_ASSIGN:
        xin = np.concatenate(
            [xq[:, J * NCOL : (J + 1) * NCOL]]
            + [xq[:, i * NBLK : (i + 1) * NBLK] if i >= 0 else zero_x for i in slots],
            axis=1,
        )
        # chunk-major DoubleRow packing: [4096, NIN] -> [16*128, 2*NIN]
        xin = (
            xin.reshape(NK, 2, 128, NIN)
            .transpose(0, 2, 1, 3)
            .reshape(NK * 128, 2 * NIN)
        )
        ddiA = np.concatenate(
            [
                db8[i * NBLK : (i + 1) * NBLK, J * NCOL : (J + 1) * NCOL]
                if i >= 0
                else zero_a
                for i in slots
            ],
            axis=1,
        )
        ddiB = np.concatenate(
            [
                db16[J * NCOL : (J + 1) * NCOL, i * NBLK : (i + 1) * NBLK]
                if i >= 0
                else zero_b
                for i in slots
            ],
            axis=1,
        )
        p = np.arange(128, dtype=np.float32)[:, None]
        thr = np.concatenate(
            [
                p + np.float32(i * NBLK - J * NCOL)
                if i >= 0
                else np.full((128, 1), 1e9, np.float32)
                for i in slots
            ],
            axis=1,
        )
        maps.append(
            {
                "xin": np.ascontiguousarray(xin),
                "ddiA": np.ascontiguousarray(ddiA),
                "ddiB": np.ascontiguousarray(ddiB),
                "thr": np.ascontiguousarray(thr),
            }
        )
    return maps


def kernel(drug_probs, ddi_matrix, **_run_kwargs):
    from concourse.bass_utils import run_bass_kernel_spmd

    if "nc" not in _CACHE:
        _CACHE["nc"] = _build()
    nc = _CACHE["nc"]

    maps = _in_maps(np.asarray(drug_probs), np.asarray(ddi_matrix))
    res = run_bass_kernel_spmd(nc, maps, list(range(8)), **_run_kwargs)
    _CACHE["last_result"] = res

    gsum = 0.0
    msum = 0.0
    for core_out in res.results:
        o = core_out["out"].astype(np.float64)
        gsum += o[:, :NSLOT].sum()
        msum += o[:, NSLOT:].sum()
    normalizer = max(msum, 1.0)
    return np.asarray(gsum / (B * normalizer), dtype=np.float32)


# revision 5
# speedup vs baseline: 2.7200x; 1.4323x over previous
"""DDI regularizer loss kernel for 8 Trainium2 NeuronCores.

reference semantics:
    b = (ddi > 0); S = max(b, b.T) with zero diagonal; U = triu(S, k=1)
    normalizer = max(U.sum(), 1.0)
    xu = drug_probs @ U; penalties = sum(xu * drug_probs, axis=1) / normalizer
    return penalties.mean()

Identity used here:
    mean_i(x_i^T U x_i) = <U, X^T X> / B
so the kernel computes G = X^T X only on upper-triangular 128x512 tiles
(contraction over the batch is the natural PE layout), masks each G tile with
U's tile (built on device from ddi slices) and reduces.  40 real tiles + 8
dummy slots are distributed 6-per-core across the 8 cores; each core returns
per-partition partial sums of (U*G) and of U, and the host combines 8 tiny
vectors into the final scalar.

The matmuls run in fp8 e5m2 with DoubleRow packing (two 128-row batch chunks
per matmul, fp32 PSUM accumulation).  Pipeline (v2):
  - X stream on the sync HWDGE queue, host-packed so every chunk is one
    2560B descriptor per partition; a few warmup matmuls on a memset tile
    raise the PE HAM clock before real data lands.
  - mirror ddi blocks arrive TRANSPOSED via the xbar DMA-transpose path on
    the scalar HWDGE queue (no PE transposes, no PSUM staging); the A-side
    ddi rides a casting SWDGE (gpsimd) DMA as fp8-over-the-wire -> bf16.
  - binarize is scalar-engine Sign (-1/0/1); the DVE combine
    mask = relu(max(signA, signB^T)) * sel fixes the -1 case in one fused
    scalar_tensor_tensor op.  The normalizer partial sum(U) is a scalar
    engine Copy-with-accum.  DVE never touches fp8 and gpsimd never runs
    elementwise ops (both are ~16x slow paths).
"""

import sys

for _p in ("/opt/trn_rl_repo", "/root/.axon_site/_ro/trn_rl_repo"):
    if _p not in sys.path:
        sys.path.insert(0, _p)

import numpy as np
import ml_dtypes

B, D = 4096, 2048
NBLK = 128  # lhs row-block width
NCOL = 512  # rhs col-block width
NSLOT = 6  # tile slots per core
NWARM = 3  # PE clock warmup matmuls
NK = B // 256  # two 128-row chunks per DoubleRow matmul

# (J, [row-block indices; -1 = dummy slot]) per core.  Tile (i, J) covers
# G[128i:128i+128, 512J:512J+512]; it exists iff i <= 4J+3 (touches the
# strict upper triangle).
CORE_ASSIGN = [
    (3, [0, 1, 2, 3, 4, 5]),
    (3, [6, 7, 8, 9, 10, 11]),
    (3, [12, 13, 14, 15, -1, -1]),
    (2, [0, 1, 2, 3, 4, 5]),
    (2, [6, 7, 8, 9, 10, 11]),
    (1, [0, 1, 2, 3, 4, 5]),
    (1, [6, 7, -1, -1, -1, -1]),
    (0, [0, 1, 2, 3, -1, -1]),
]

NIN = NCOL + NBLK * NSLOT  # 1280 columns in the merged X input

_CACHE = {}


def _build():
    import concourse.bass as bass
    import concourse.mybir as mybir
    from concourse import bacc
    from concourse.tile import TileContext

    f32 = mybir.dt.float32
    bf16 = mybir.dt.bfloat16
    fp8 = mybir.dt.float8e5
    op = mybir.AluOpType
    act = mybir.ActivationFunctionType

    nc = bacc.Bacc("TRN2", target_bir_lowering=False, debug=False, num_devices=8)

    # xin: chunk-major DoubleRow layout - row 128k+p holds the two batch rows
    # 256k+p and 256k+128+p back to back (2560 contiguous bytes / partition).
    xin_d = nc.dram_tensor("xin", [NK * 128, 2 * NIN], fp8, kind="ExternalInput")
    ddiA_d = nc.dram_tensor("ddiA", [NBLK, NCOL * NSLOT], fp8, kind="ExternalInput")
    # mirror blocks, host-transposed to [128, 512] per slot (pure layout)
    ddiB_d = nc.dram_tensor("ddiB", [NBLK, NCOL * NSLOT], fp8, kind="ExternalInput")
    thr_d = nc.dram_tensor("thr", [128, NSLOT], f32, kind="ExternalInput")
    out_d = nc.dram_tensor("out", [128, 2 * NSLOT], f32, kind="ExternalOutput")

    with TileContext(nc) as tc:
        with (
            tc.tile_pool(name="const", bufs=1) as cpool,
            tc.tile_pool(name="masks", bufs=NSLOT) as mpool,
            tc.tile_pool(name="io", bufs=10) as iopool,
            tc.tile_pool(name="psum", bufs=NSLOT, space="PSUM") as ppool,
            tc.tile_pool(name="tpp", bufs=1, space="PSUM") as tppool,
            tc.tile_pool(name="scr", bufs=3) as spool,
        ):
            # --- gpsimd: warmup source memset first (fast native op, lets
            # the PE clock-warmup start ~6.5us), then iota, then the ddi/thr
            # casting SWDGE DMAs (fp8 on the wire -> bf16 in SBUF) ---
            wsrc = cpool.tile([128, 2, NCOL + NBLK], fp8, tag="wsrc")
            nc.gpsimd.memset(wsrc, 0.0)
            iota = cpool.tile([128, NCOL], f32, tag="iota")
            nc.gpsimd.iota(
                iota,
                pattern=[[1, NCOL]],
                base=0,
                channel_multiplier=0,
                allow_small_or_imprecise_dtypes=True,
            )
            ddiA_sb = cpool.tile([NBLK, NCOL * NSLOT], bf16, tag="ddiA")
            nc.gpsimd.dma_start(out=ddiA_sb, in_=ddiA_d.ap())
            ddiBT_raw = cpool.tile([NBLK, NCOL * NSLOT], bf16, tag="ddiBTr")
            nc.gpsimd.dma_start(out=ddiBT_raw, in_=ddiB_d.ap())
            thr_sb = cpool.tile([128, NSLOT], f32, tag="thr")
            nc.gpsimd.dma_start(out=thr_sb, in_=thr_d.ap())

            # --- PE clock warmup on the memset tile ---
            wps = tppool.tile([128, NCOL], f32, tag="tp", name="warm")
            for w in range(NWARM):
                nc.tensor.matmul(
                    out=wps,
                    lhsT=wsrc[:, :, NCOL : NCOL + NBLK],
                    rhs=wsrc[:, :, 0:NCOL],
                    start=True,
                    stop=True,
                    perf_mode=mybir.MatmulPerfMode.DoubleRow,
                )

            # binarize to sign (-1/0/1) on the scalar engine
            binA = cpool.tile([NBLK, NCOL * NSLOT], bf16, tag="binA")
            nc.scalar.activation(out=binA, in_=ddiA_sb, func=act.Sign)
            binBT = cpool.tile([NBLK, NCOL * NSLOT], bf16, tag="binBT")
            nc.scalar.activation(out=binBT, in_=ddiBT_raw, func=act.Sign)

            # --- G tiles: accumulating matmuls, k-outer so the X stream is
            # consumed strictly in order ---
            psums = [
                ppool.tile([128, NCOL], f32, tag="gps", name=f"gps{t}")
                for t in range(NSLOT)
            ]
            xin_ap = xin_d.ap().rearrange("(k p) (i c) -> k p i c", p=128, i=2)
            for k in range(NK):
                xt = iopool.tile([128, 2, NIN], fp8, tag="xt")
                nc.sync.dma_start(out=xt, in_=xin_ap[k])
                for t in range(NSLOT):
                    c0 = NCOL + t * NBLK
                    nc.tensor.matmul(
                        out=psums[t],
                        lhsT=xt[:, :, c0 : c0 + NBLK],
                        rhs=xt[:, :, 0:NCOL],
                        start=(k == 0),
                        stop=(k == NK - 1),
                        perf_mode=mybir.MatmulPerfMode.DoubleRow,
                    )

            # masks on DVE, overlapped with the matmul phase:
            # U_tile = relu(max(signA, signB^T)) * (col > row)
            out_sb = cpool.tile([128, 2 * NSLOT], f32, tag="out")
            masks = []
            for t in range(NSLOT):
                sel = spool.tile([128, NCOL], bf16, tag="sel")
                nc.vector.tensor_scalar(
                    out=sel, in0=iota, scalar1=thr_sb[:, t : t + 1],
                    scalar2=None, op0=op.is_gt,
                )
                mraw = spool.tile([128, NCOL], bf16, tag="mraw")
                nc.vector.tensor_tensor(
                    out=mraw, in0=binA[:, t * NCOL : (t + 1) * NCOL],
                    in1=binBT[:, t * NCOL : (t + 1) * NCOL], op=op.max,
                )
                mask = mpool.tile([128, NCOL], bf16, tag="mask")
                nc.vector.scalar_tensor_tensor(
                    out=mask, in0=mraw, scalar=0.0, in1=sel,
                    op0=op.max, op1=op.mult,
                )
                masks.append(mask)
                # normalizer partial sum(U) on the scalar engine
                mjunk = spool.tile([128, NCOL], bf16, tag="mjunk")
                nc.scalar.activation(
                    out=mjunk, in_=mask, func=act.Copy,
                    accum_out=out_sb[:, NSLOT + t : NSLOT + t + 1],
                )

            # --- masked reduction: sum(G * mask), one fused op per slot ---
            for t in range(NSLOT):
                gjunk = spool.tile([128, NCOL], f32, tag="gjunk")
                nc.vector.scalar_tensor_tensor(
                    out=gjunk, in0=psums[t], scalar=1.0, in1=masks[t],
                    op0=op.mult, op1=op.mult,
                    accum_out=out_sb[:, t : t + 1],
                )

            nc.gpsimd.dma_start(out=out_d.ap(), in_=out_sb)

    nc.compile()
    return nc


def _in_maps(drug_probs, ddi_matrix):
    fp8 = ml_dtypes.float8_e5m2
    xq = drug_probs.astype(fp8)
    db8 = ddi_matrix.astype(fp8)
    zero_x = np.zeros((B, NBLK), dtype=fp8)
    zero_a = np.zeros((NBLK, NCOL), dtype=fp8)
    zero_b = np.zeros((NBLK, NCOL), dtype=fp8)
    maps = []
    for J, slots in CORE_ASSIGN:
        xin = np.concatenate(
            [xq[:, J * NCOL : (J + 1) * NCOL]]
            + [xq[:, i * NBLK : (i + 1) * NBLK] if i >= 0 else zero_x for i in slots],
            axis=1,
        )
        # chunk-major DoubleRow packing: [4096, NIN] -> [16*128, 2*NIN]
        xin = (
            xin.reshape(NK, 2, 128, NIN)
            .transpose(0, 2, 1, 3)
            .reshape(NK * 128, 2 * NIN)
        )
        ddiA = np.concatenate(
            [
                db8[i * NBLK : (i + 1) * NBLK, J * NCOL : (J + 1) * NCOL]
                if i >= 0
                else zero_a
                for i in slots
            ],
            axis=1,
        )
        ddiB = np.concatenate(
            [
                db8[J * NCOL : (J + 1) * NCOL, i * NBLK : (i + 1) * NBLK].T
                if i >= 0
                else zero_b
                for i in slots
            ],
            axis=1,
        )
        p = np.arange(128, dtype=np.float32)[:, None]
        thr = np.concatenate(
            [
                p + np.float32(i * NBLK - J * NCOL)
                if i >= 0
                else np.full((128, 1), 1e9, np.float32)
                for i in slots
            ],
            axis=1,
        )
        maps.append(
            {
                "xin": np.ascontiguousarray(xin),
                "ddiA": np.ascontiguousarray(ddiA),
                "ddiB": np.ascontiguousarray(ddiB),
                "thr": np.ascontiguousarray(thr),
            }
        )
    return maps


def kernel(drug_probs, ddi_matrix, **_run_kwargs):
    from concourse.bass_utils import run_bass_kernel_spmd

    if "nc" not in _CACHE:
        _CACHE["nc"] = _build()
    nc = _CACHE["nc"]

    maps = _in_maps(np.asarray(drug_probs), np.asarray(ddi_matrix))
    res = run_bass_kernel_spmd(nc, maps, list(range(8)), **_run_kwargs)
    _CACHE["last_result"] = res

    gsum = 0.0
    msum = 0.0
    for core_out in res.results:
        o = core_out["out"].astype(np.float64)
        gsum += o[:, :NSLOT].sum()
        msum += o[:, NSLOT:].sum()
    normalizer = max(msum, 1.0)
    return np.asarray(gsum / (B * normalizer), dtype=np.float32)


# revision 8
# speedup vs baseline: 2.7950x; 1.0276x over previous
"""DDI regularizer loss kernel for 8 Trainium2 NeuronCores.

reference semantics:
    b = (ddi > 0); S = max(b, b.T) with zero diagonal; U = triu(S, k=1)
    normalizer = max(U.sum(), 1.0)
    xu = drug_probs @ U; penalties = sum(xu * drug_probs, axis=1) / normalizer
    return penalties.mean()

Identity used here:
    mean_i(x_i^T U x_i) = <U, X^T X> / B
so the kernel computes G = X^T X only on upper-triangular 128x512 tiles
(contraction over the batch is the natural PE layout), masks each G tile with
U's tile (built on device from ddi slices) and reduces.  40 real tiles + 8
dummy slots are distributed 6-per-core across the 8 cores; each core returns
per-partition partial sums of (U*G) and of U, and the host combines 8 tiny
vectors into the final scalar.

The matmuls run in fp8 e5m2 with DoubleRow packing (two 128-row batch chunks
per matmul, fp32 PSUM accumulation).  Pipeline (v2):
  - X stream on the sync HWDGE queue, host-packed so every chunk is one
    2560B descriptor per partition; a few warmup matmuls on a memset tile
    raise the PE HAM clock before real data lands.
  - mirror ddi blocks arrive TRANSPOSED via the xbar DMA-transpose path on
    the scalar HWDGE queue (no PE transposes, no PSUM staging); the A-side
    ddi rides a casting SWDGE (gpsimd) DMA as fp8-over-the-wire -> bf16.
  - binarize is scalar-engine Sign (-1/0/1); the DVE combine
    mask = relu(max(signA, signB^T)) * sel fixes the -1 case in one fused
    scalar_tensor_tensor op.  The normalizer partial sum(U) is a scalar
    engine Copy-with-accum.  DVE never touches fp8 and gpsimd never runs
    elementwise ops (both are ~16x slow paths).
"""

import sys

for _p in ("/opt/trn_rl_repo", "/root/.axon_site/_ro/trn_rl_repo"):
    if _p not in sys.path:
        sys.path.insert(0, _p)

import numpy as np
import ml_dtypes

B, D = 4096, 2048
NBLK = 128  # lhs row-block width
NCOL = 512  # rhs col-block width
NSLOT = 6  # tile slots per core
NWARM = 25  # PE clock warmup matmuls (N=64 bridge)
NK = B // 256  # two 128-row chunks per DoubleRow matmul

# (J, [row-block indices; -1 = dummy slot]) per core.  Tile (i, J) covers
# G[128i:128i+128, 512J:512J+512]; it exists iff i <= 4J+3 (touches the
# strict upper triangle).
CORE_ASSIGN = [
    (3, [0, 1, 2, 3, 4, 5]),
    (3, [6, 7, 8, 9, 10, 11]),
    (3, [12, 13, 14, 15, -1, -1]),
    (2, [0, 1, 2, 3, 4, 5]),
    (2, [6, 7, 8, 9, 10, 11]),
    (1, [0, 1, 2, 3, 4, 5]),
    (1, [6, 7, -1, -1, -1, -1]),
    (0, [0, 1, 2, 3, -1, -1]),
]

NIN = NCOL + NBLK * NSLOT  # 1280 columns in the merged X input

_CACHE = {}


def _build():
    import concourse.bass as bass
    import concourse.mybir as mybir
    from concourse import bacc
    from concourse.tile import TileContext

    f32 = mybir.dt.float32
    bf16 = mybir.dt.bfloat16
    fp8 = mybir.dt.float8e5
    op = mybir.AluOpType
    act = mybir.ActivationFunctionType

    nc = bacc.Bacc("TRN2", target_bir_lowering=False, debug=False, num_devices=8)

    # xin: chunk-major DoubleRow layout - row 128k+p holds the two batch rows
    # 256k+p and 256k+128+p back to back (2560 contiguous bytes / partition).
    xin_d = nc.dram_tensor("xin", [NK * 128, 2 * NIN], fp8, kind="ExternalInput")
    ddiA_d = nc.dram_tensor("ddiA", [NBLK, NCOL * NSLOT], fp8, kind="ExternalInput")
    # mirror blocks, host-transposed to [128, 512] per slot (pure layout)
    ddiB_d = nc.dram_tensor("ddiB", [NBLK, NCOL * NSLOT], fp8, kind="ExternalInput")
    thr_d = nc.dram_tensor("thr", [128, NSLOT], f32, kind="ExternalInput")
    out_d = nc.dram_tensor("out", [128, 2 * NSLOT], f32, kind="ExternalOutput")

    with TileContext(nc) as tc:
        with (
            tc.tile_pool(name="const", bufs=1) as cpool,
            tc.tile_pool(name="masks", bufs=NSLOT) as mpool,
            tc.tile_pool(name="io", bufs=16) as iopool,
            tc.tile_pool(name="psum", bufs=NSLOT, space="PSUM") as ppool,
            tc.tile_pool(name="tpp", bufs=1, space="PSUM") as tppool,
            tc.tile_pool(name="scr", bufs=6) as spool,
        ):
            # --- gpsimd: tiny warmup-source memset first, then the ddi/thr
            # casting SWDGE DMAs (fp8 on the wire -> bf16 in SBUF), then iota ---
            wsrc = cpool.tile([128, 2, 192], fp8, tag="wsrc")
            nc.gpsimd.memset(wsrc, 0.0)
            ddiA_sb = cpool.tile([NBLK, NCOL * NSLOT], bf16, tag="ddiA")
            nc.gpsimd.dma_start(out=ddiA_sb, in_=ddiA_d.ap())
            ddiBT_raw = cpool.tile([NBLK, NCOL * NSLOT], bf16, tag="ddiBTr")
            nc.gpsimd.dma_start(out=ddiBT_raw, in_=ddiB_d.ap())
            thr_sb = cpool.tile([128, NSLOT], f32, tag="thr")
            nc.gpsimd.dma_start(out=thr_sb, in_=thr_d.ap())
            iota = cpool.tile([128, NCOL], f32, tag="iota")
            nc.gpsimd.iota(
                iota,
                pattern=[[1, NCOL]],
                base=0,
                channel_multiplier=0,
                allow_small_or_imprecise_dtypes=True,
            )

            # --- PE HAM clock warmup: a bridge of cheap N=64 matmuls keeps
            # the PE busy from engine boot until real chunks land, so the
            # 4096-cycle activity window flips to full clock early ---
            wps = tppool.tile([128, 64], f32, tag="tp", name="warm")
            for w in range(NWARM):
                nc.tensor.matmul(
                    out=wps,
                    lhsT=wsrc[:, :, 64 : 64 + NBLK],
                    rhs=wsrc[:, :, 0:64],
                    start=True,
                    stop=True,
                    perf_mode=mybir.MatmulPerfMode.DoubleRow,
                )

            # --- G tiles: accumulating matmuls, k-outer so the X stream is
            # consumed strictly in order ---
            psums = [
                ppool.tile([128, NCOL], f32, tag="gps", name=f"gps{t}")
                for t in range(NSLOT)
            ]
            xin_ap = xin_d.ap().rearrange("(k p) (i c) -> k p i c", p=128, i=2)
            for k in range(NK):
                xt = iopool.tile([128, 2, NIN], fp8, tag="xt")
                eng = nc.sync if k % 2 == 0 else nc.scalar
                eng.dma_start(out=xt, in_=xin_ap[k])
                for t in range(NSLOT):
                    c0 = NCOL + t * NBLK
                    nc.tensor.matmul(
                        out=psums[t],
                        lhsT=xt[:, :, c0 : c0 + NBLK],
                        rhs=xt[:, :, 0:NCOL],
                        start=(k == 0),
                        stop=(k == NK - 1),
                        perf_mode=mybir.MatmulPerfMode.DoubleRow,
                    )

            # binarize to sign (-1/0/1) on the scalar engine
            binA = cpool.tile([NBLK, NCOL * NSLOT], bf16, tag="binA")
            nc.scalar.activation(out=binA, in_=ddiA_sb, func=act.Sign)
            binBT = cpool.tile([NBLK, NCOL * NSLOT], bf16, tag="binBT")
            nc.scalar.activation(out=binBT, in_=ddiBT_raw, func=act.Sign)

            # masks on DVE, overlapped with the matmul phase:
            # U_tile = relu(max(signA, signB^T)) * (col > row)
            out_sb = cpool.tile([128, 2 * NSLOT], f32, tag="out")
            masks = []
            for t in range(NSLOT):
                sel = spool.tile([128, NCOL], bf16, tag="sel")
                nc.vector.tensor_scalar(
                    out=sel, in0=iota, scalar1=thr_sb[:, t : t + 1],
                    scalar2=None, op0=op.is_gt,
                )
                mraw = spool.tile([128, NCOL], bf16, tag="mraw")
                nc.vector.tensor_tensor(
                    out=mraw, in0=binA[:, t * NCOL : (t + 1) * NCOL],
                    in1=binBT[:, t * NCOL : (t + 1) * NCOL], op=op.max,
                )
                mask = mpool.tile([128, NCOL], bf16, tag="mask")
                nc.vector.scalar_tensor_tensor(
                    out=mask, in0=mraw, scalar=0.0, in1=sel,
                    op0=op.max, op1=op.mult,
                )
                masks.append(mask)
                # normalizer partial sum(U) on the scalar engine
                mjunk = spool.tile([128, NCOL], bf16, tag="mjunk")
                nc.scalar.activation(
                    out=mjunk, in_=mask, func=act.Copy,
                    accum_out=out_sb[:, NSLOT + t : NSLOT + t + 1],
                )

            # --- masked reduction: sum(G * mask), one fused op per slot ---
            for t in range(NSLOT):
                gjunk = spool.tile([128, NCOL], f32, tag="gjunk")
                nc.vector.scalar_tensor_tensor(
                    out=gjunk, in0=psums[t], scalar=1.0, in1=masks[t],
                    op0=op.mult, op1=op.mult,
                    accum_out=out_sb[:, t : t + 1],
                )

            nc.sync.dma_start(out=out_d.ap(), in_=out_sb)

    nc.compile()
    return nc


def _in_maps(drug_probs, ddi_matrix):
    fp8 = ml_dtypes.float8_e5m2
    xq = drug_probs.astype(fp8)
    db8 = ddi_matrix.astype(fp8)
    zero_x = np.zeros((B, NBLK), dtype=fp8)
    zero_a = np.zeros((NBLK, NCOL), dtype=fp8)
    zero_b = np.zeros((NBLK, NCOL), dtype=fp8)
    maps = []
    for J, slots in CORE_ASSIGN:
        xin = np.concatenate(
            [xq[:, J * NCOL : (J + 1) * NCOL]]
            + [xq[:, i * NBLK : (i + 1) * NBLK] if i >= 0 else zero_x for i in slots],
            axis=1,
        )
        # chunk-major DoubleRow packing: [4096, NIN] -> [16*128, 2*NIN]
        xin = (
            xin.reshape(NK, 2, 128, NIN)
            .transpose(0, 2, 1, 3)
            .reshape(NK * 128, 2 * NIN)
        )
        ddiA = np.concatenate(
            [
                db8[i * NBLK : (i + 1) * NBLK, J * NCOL : (J + 1) * NCOL]
                if i >= 0
                else zero_a
                for i in slots
            ],
            axis=1,
        )
        ddiB = np.concatenate(
            [
                db8[J * NCOL : (J + 1) * NCOL, i * NBLK : (i + 1) * NBLK].T
                if i >= 0
                else zero_b
                for i in slots
            ],
            axis=1,
        )
        p = np.arange(128, dtype=np.float32)[:, None]
        thr = np.concatenate(
            [
                p + np.float32(i * NBLK - J * NCOL)
                if i >= 0
                else np.full((128, 1), 1e9, np.float32)
                for i in slots
            ],
            axis=1,
        )
        maps.append(
            {
                "xin": np.ascontiguousarray(xin),
                "ddiA": np.ascontiguousarray(ddiA),
                "ddiB": np.ascontiguousarray(ddiB),
                "thr": np.ascontiguousarray(thr),
            }
        )
    return maps


def kernel(drug_probs, ddi_matrix, **_run_kwargs):
    from concourse.bass_utils import run_bass_kernel_spmd

    if "nc" not in _CACHE:
        _CACHE["nc"] = _build()
    nc = _CACHE["nc"]

    maps = _in_maps(np.asarray(drug_probs), np.asarray(ddi_matrix))
    res = run_bass_kernel_spmd(nc, maps, list(range(8)), **_run_kwargs)
    _CACHE["last_result"] = res

    gsum = 0.0
    msum = 0.0
    for core_out in res.results:
        o = core_out["out"].astype(np.float64)
        gsum += o[:, :NSLOT].sum()
        msum += o[:, NSLOT:].sum()
    normalizer = max(msum, 1.0)
    return np.asarray(gsum / (B * normalizer), dtype=np.float32)
